# revision 1
# baseline (speedup 1.0000x reference)
"""Trainium2 Bass kernel for a 2-layer GATv2 GNN (nn_ComponentGNN), v2.

Strategy vs v1: gather narrow bf16 node-feature rows (256B) with
dma_gather(transpose=True) and apply the GAT linear transforms on-chip with
bf16 matmuls; per-edge xr comes from a one-hot-transpose matmul against a
per-block xr tile (no DRAM gather); messages use the z-form trick
(scatter z*w, correct with xr*den after normalize); leaky-relu runs on the
ACT engine (Prelu alpha), the per-head logit reduce on the Pool engine.

Sharding: nodes (and their in-edges grouped by destination block) across 8
cores; weights replicated; bf16 node tables AllGather'd between layers;
per-graph pooled sums AllReduced.
"""
import math
import sys

import numpy as np

sys.path.insert(0, "/opt/trn_rl_repo")

NEG_SLOPE = 0.2
EPS = 1e-5
HEADS = 4
HID = 64
F_IN = 256
NCORES = 8
NQ = 4          # src-table quarters (dma_gather idx is int16)
CB = 4          # dst blocks per chunk (gather granularity)


def as_bf16(x):
    import jax.numpy as jnp
    return np.asarray(jnp.asarray(np.asarray(x, np.float32), jnp.bfloat16))


# ---------------------------------------------------------------- config ----
class Cfg:
    def __init__(self, N, E, G, KQ):
        self.N, self.E, self.G = N, E, G
        assert N % NCORES == 0
        self.NS_REAL = N // NCORES
        self.NS = ((self.NS_REAL + 127) // 128) * 128
        self.NB = self.NS // 128
        self.NTAB = self.NS * NCORES
        assert self.NTAB % NQ == 0
        self.QROWS = self.NTAB // NQ
        assert self.QROWS <= 32768
        self.KQ = KQ
        self.CAP = KQ * 128
        # chunks of CB dst blocks
        self.CHUNKS = []
        b = 0
        while b < self.NB:
            self.CHUNKS.append(min(CB, self.NB - b))
            b += CB
        # slot base of each chunk
        self.CHUNK_SLOT0 = np.concatenate(
            [[0], np.cumsum([NQ * c * self.CAP for c in self.CHUNKS])]).astype(np.int64)
        self.S = int(self.CHUNK_SLOT0[-1])
        assert self.S == self.NB * NQ * self.CAP


def compute_kq(inputs, cfg_like=None):
    edge_index = np.asarray(inputs["edge_index"])
    N = int(np.asarray(inputs["x"]).shape[0])
    NS_REAL = N // NCORES
    NS = ((NS_REAL + 127) // 128) * 128
    NB = NS // 128
    QROWS = NS * NCORES // NQ
    src = edge_index[0].astype(np.int64)
    dst = edge_index[1].astype(np.int64)
    core = dst // NS_REAL
    dstl = dst % NS_REAL
    blk = dstl // 128
    srow = (src // NS_REAL) * NS + (src % NS_REAL)
    q = srow // QROWS
    keys = (core * NQ + q) * NB + blk
    cnt = np.bincount(keys, minlength=NCORES * NQ * NB)
    return int(math.ceil(cnt.max() / 128))


def host_prep(inputs, cfg):
    c = cfg
    edge_index = np.asarray(inputs["edge_index"])
    batch = np.asarray(inputs["batch"])
    src = edge_index[0].astype(np.int64)
    dst = edge_index[1].astype(np.int64)

    core = dst // c.NS_REAL
    dstl = dst % c.NS_REAL
    blk = dstl // 128
    srow = (src // c.NS_REAL) * c.NS + (src % c.NS_REAL)
    q = srow // c.QROWS
    srcq = srow % c.QROWS

    ci = blk // CB                      # chunk index
    j = blk % CB                        # block within chunk
    cbc = np.minimum(CB, c.NB - ci * CB)
    slot_base = c.CHUNK_SLOT0[ci] + (q * cbc + j) * c.CAP

    order = np.lexsort((slot_base, core))
    so_core = core[order]
    so_base = slot_base[order]
    cell_key = so_core.astype(np.int64) * c.S + so_base
    chg = np.empty(len(cell_key), dtype=bool)
    chg[0] = True
    chg[1:] = cell_key[1:] != cell_key[:-1]
    seg_start = np.maximum.accumulate(np.where(chg, np.arange(len(cell_key)), 0))
    slot = so_base + (np.arange(len(cell_key)) - seg_start)

    srcq_idx = np.zeros((NCORES, c.S), np.int16)
    dstrel = np.full((NCORES, c.S), -1.0, np.float32)
    srcq_idx[so_core, slot] = srcq[order].astype(np.int16)
    dstrel[so_core, slot] = (dstl[order] - blk[order] * 128).astype(np.float32)

    def wrap16(a):  # [S] int16 -> [128, S//16]
        w = a.reshape(-1, 16).T
        return np.tile(w, (8, 1)).copy()

    srcq_w = np.stack([wrap16(srcq_idx[i]) for i in range(NCORES)])
    dstrel_pm = dstrel.reshape(NCORES, c.S // 128, 128).transpose(0, 2, 1)

    batchloc = np.full((NCORES, c.NS), -1.0, np.float32)
    for i in range(NCORES):
        batchloc[i, :c.NS_REAL] = batch[i * c.NS_REAL:(i + 1) * c.NS_REAL]
    batchloc_pm = batchloc.reshape(NCORES, c.NB, 128).transpose(0, 2, 1)

    x = np.asarray(inputs["x"], np.float32)
    xT = np.zeros((NCORES, F_IN, c.NS), np.float32)
    for i in range(NCORES):
        xT[i, :, :c.NS_REAL] = x[i * c.NS_REAL:(i + 1) * c.NS_REAL].T

    f64 = lambda k: np.asarray(inputs[k], np.float64)

    def fold(bias, g, be, m, v):
        a = f64(g) / np.sqrt(f64(v) + EPS)
        B = a * (f64(bias) - f64(m)) + f64(be)
        return a, B

    A0, B0 = fold("bias0", "g0", "be0", "m0", "v0")
    A1, B1 = fold("bias1", "g1", "be1", "m1", "v1")
    A1q = A1 * 0.25

    # (c, h)-major column permutation: new col c*H + h <- old col h*ch + c.
    def chperm(ch):
        p = np.empty(HEADS * ch, np.int64)
        for h in range(HEADS):
            for cc in range(ch):
                p[cc * HEADS + h] = h * ch + cc
        return p

    P0 = chperm(16)   # L0: 64 z-cols / output dims
    P1 = chperm(64)   # L1: 256 z-cols

    # z-path weights with bias row appended ([65, W])
    def wb(Wk, bk):
        return np.concatenate([f64(Wk), f64(bk).reshape(1, -1)], axis=0)

    # --- L0 (everything in its z/output space permuted by P0)
    Wl0b = wb("Wl0", "bl0")[:, P0]                # [65, 64]
    Wr0b = wb("Wr0", "br0")[:, P0]                # [65, 64]
    A0p, B0p = A0[P0], B0[P0]
    sub0 = Wr0b * A0p[None, :]
    sub0[64, :] -= B0p
    Wr0x = np.concatenate([Wr0b, sub0], axis=1)   # [65, 128]
    att0 = f64("att0").reshape(-1)[P0].reshape(1, -1)   # [1, 64]

    # --- L1 (rows are h1 dims -> permuted by P0; z-cols permuted by P1)
    def rowperm(W65):
        out = W65.copy()
        out[:64, :] = W65[:64, :][P0, :]
        return out

    Wl1b = rowperm(wb("Wl1", "bl1"))[:, P1]       # [65, 256]
    Wr1b = rowperm(wb("Wr1", "br1"))              # [65, 256] natural cols
    Wr1h = Wr1b.reshape(65, HEADS, HID).sum(axis=1)   # sum over heads [65, 64]
    sub1 = Wr1h * A1q[None, :]
    sub1[64, :] -= B1
    Wr1x = np.concatenate([Wr1b[:, P1], sub1], axis=1)  # [65, 320]
    att1 = f64("att1").reshape(-1)[P1].reshape(1, -1)   # [1, 256]

    rep = dict(
        W_in=as_bf16(inputs["W_in"]),
        b_in_c=np.asarray(inputs["b_in"], np.float32).reshape(-1, 1),
        Wl0b=as_bf16(Wl0b), Wr0x=as_bf16(Wr0x),
        Wl1b=as_bf16(Wl1b), Wr1x=as_bf16(Wr1x),
        att0_r=as_bf16(att0), att1_r=as_bf16(att1),
        A0_r=as_bf16(A0p.reshape(1, -1)), A1q_r=as_bf16(A1q.reshape(1, -1)),
        io128_r=as_bf16(np.arange(128, dtype=np.float32).reshape(1, -1)),
        iotaG_r=as_bf16(np.arange(cfg.G, dtype=np.float32).reshape(1, -1)),
        identb=as_bf16(np.eye(128, dtype=np.float32)),
        ident32=np.eye(128, dtype=np.float32),
        Wc1=np.asarray(inputs["Wc1"], np.float32),
        bc1_c=np.asarray(inputs["bc1"], np.float32).reshape(-1, 1),
        Wc2=np.asarray(inputs["Wc2"], np.float32),
        bc2_c=np.asarray(inputs["bc2"], np.float32).reshape(-1, 1),
    )
    per_core = [dict(
        xT=as_bf16(xT[i]), srcq_w=srcq_w[i],
        dstrel=np.ascontiguousarray(dstrel_pm[i], np.float32),
        batchloc=as_bf16(batchloc_pm[i]),
        **rep,
    ) for i in range(NCORES)]
    return per_core


# ---------------------------------------------------------------- device ----
def build_program(cfg, reps=1, sim_mode=False):
    from contextlib import ExitStack
    import concourse.bass as bass
    import concourse.tile as tile
    from concourse import bacc, mybir

    c = cfg
    f32 = mybir.dt.float32
    bf16 = mybir.dt.bfloat16
    i16 = mybir.dt.int16
    AF = mybir.ActivationFunctionType
    OP = mybir.AluOpType

    nc = bacc.Bacc("TRN2", target_bir_lowering=False, debug=False,
                   num_devices=NCORES)

    def din(name, shape, dt=f32):
        return nc.dram_tensor(name, list(shape), dt, kind="ExternalInput").ap()

    xT = din("xT", [F_IN, c.NS], bf16)
    srcq_w = din("srcq_w", [128, c.S // 16], i16)
    dstrel_d = din("dstrel", [128, c.S // 128])
    batchloc_d = din("batchloc", [128, c.NB], bf16)
    W_in = din("W_in", [F_IN, HID], bf16)
    b_in_c = din("b_in_c", [HID, 1])
    Wl0b_d = din("Wl0b", [65, HID], bf16)
    Wr0x_d = din("Wr0x", [65, 2 * HID], bf16)
    Wl1b_d = din("Wl1b", [65, 256], bf16)
    Wr1x_d = din("Wr1x", [65, 320], bf16)
    att0_r = din("att0_r", [1, HID], bf16)
    att1_r = din("att1_r", [1, 256], bf16)
    A0_r = din("A0_r", [1, HID], bf16)
    A1q_r = din("A1q_r", [1, HID], bf16)
    io128_r = din("io128_r", [1, 128], bf16)
    iotaG_r = din("iotaG_r", [1, c.G], bf16)
    identb_d = din("identb", [128, 128], bf16)
    ident32_d = din("ident32", [128, 128])
    Wc1 = din("Wc1", [HID, 32]); bc1_c = din("bc1_c", [32, 1])
    Wc2 = din("Wc2", [32, 2]); bc2_c = din("bc2_c", [2, 1])

    out_t = nc.dram_tensor("out_t", [2, c.G], f32, kind="ExternalOutput").ap()

    def dram(name, shape, dt=f32, shared=False):
        return nc.dram_tensor(name, list(shape), dt, kind="Internal",
                              addr_space="Shared" if shared else "Local").ap()

    htab0_sh = dram("htab0_sh", [c.NS, 128], bf16)
    htab0_full = dram("htab0_full", [c.NTAB, 128], bf16, shared=True)
    htab1_sh = dram("htab1_sh", [c.NS, 128], bf16)
    htab1_full = dram("htab1_full", [c.NTAB, 128], bf16, shared=True)
    pool_in = dram("pool_in", [c.G, HID + 1])
    pool_out = dram("pool_out", [c.G, HID + 1], shared=True)

    RG = [list(range(NCORES))]

    def bc(ap, dims):
        pat = []
        for ax, size in dims:
            if ax is None:
                pat.append([0, size])
            else:
                st, sz = ap.ap[ax]
                assert sz == size, (ap.ap, ax, size)
                pat.append([st, size])
        return bass.AP(tensor=ap.tensor, offset=ap.offset, ap=pat)

    with tile.TileContext(nc) as tc, ExitStack() as ctx:
        singles = ctx.enter_context(tc.tile_pool(name="singles", bufs=1))
        _cn = [0]

        def load_const(ap_d, shape, dt=f32):
            _cn[0] += 1
            t = singles.tile(list(shape), dt, name=f"c{_cn[0]}")
            nc.sync.dma_start(out=t[:], in_=ap_d)
            return t

        def load_row128(ap_d, width, dt=bf16):
            _cn[0] += 1
            t = singles.tile([128, width], dt, name=f"c{_cn[0]}")
            src = bass.AP(tensor=ap_d.tensor, offset=ap_d.offset,
                          ap=[[0, 128], [1, width]])
            nc.sync.dma_start(out=t[:], in_=src)
            return t

        W_in_h = []
        for kh in range(F_IN // 128):
            t = singles.tile([128, HID], bf16, name=f"Win{kh}")
            nc.sync.dma_start(out=t[:], in_=W_in[kh * 128:(kh + 1) * 128, :])
            W_in_h.append(t)
        b_in_sb = load_const(b_in_c, [HID, 1])
        Wl0b_sb = load_const(Wl0b_d, [65, HID], bf16)
        Wr0x_sb = load_const(Wr0x_d, [65, 2 * HID], bf16)
        Wl1b_sb = load_const(Wl1b_d, [65, 256], bf16)
        Wr1x_sb = load_const(Wr1x_d, [65, 320], bf16)
        att0_sb = load_row128(att0_r, HID)
        att1_sb = load_row128(att1_r, 256)
        A0_sb = load_row128(A0_r, HID)
        A1q_sb = load_row128(A1q_r, HID)
        io128_sb = load_row128(io128_r, 128)
        ioG_sb = load_row128(iotaG_r, c.G)
        identb_sb = load_const(identb_d, [128, 128], bf16)
        ident32_sb = load_const(ident32_d, [128, 128])
        Wc1_sb = load_const(Wc1, [HID, 32]); bc1_sb = load_const(bc1_c, [32, 1])
        Wc2_sb = load_const(Wc2, [32, 2]); bc2_sb = load_const(bc2_c, [2, 1])
        batchloc_sb = load_const(batchloc_d, [128, c.NB], bf16)
        dstrel_sb = load_const(dstrel_d, [128, c.S // 128])

        # resident transposed node features [65, NS] bf16 (row 64 = ones)
        h0Te = singles.tile([65, c.NS], bf16, name="h0Te")
        h1Te = singles.tile([65, c.NS], bf16, name="h1Te")
        htab_sb = singles.tile([128, c.NB, 128], bf16, name="htab_sb")
        nc.vector.memset(h0Te[64:65, :], 1.0)
        nc.vector.memset(h1Te[64:65, :], 1.0)
        nc.vector.memset(htab_sb[:, :, 64:65], 1.0)
        nc.vector.memset(htab_sb[:, :, 65:128], 0.0)

        # ---------------- P1: h0Te = relu(W_in^T x + b), bf16 ---------------
        with tc.tile_pool(name="p1", bufs=3) as p1, \
             tc.tile_pool(name="p1ps", bufs=2, space="PSUM") as p1ps:
            n0 = 0
            while n0 < c.NS:
                nw = min(512, c.NS - n0)
                ps = p1ps.tile([HID, 512], f32)
                for kh in range(F_IN // 128):
                    xt_t = p1.tile([128, 512], bf16, name="xt")
                    nc.sync.dma_start(out=xt_t[:, :nw],
                                      in_=xT[kh * 128:(kh + 1) * 128, n0:n0 + nw])
                    nc.tensor.matmul(ps[:, :nw], W_in_h[kh][:], xt_t[:, :nw],
                                     start=(kh == 0), stop=(kh == F_IN // 128 - 1))
                nc.scalar.activation(h0Te[:64, n0:n0 + nw], ps[:, :nw], AF.Relu,
                                     bias=b_in_sb[:], scale=1.0)
                n0 += nw

        # ---------------- h0 table shard (node-major bf16) + AllGather ------
        def build_tab(hTe, tab_sh, tab_full):
            with tc.tile_pool(name="tb_ps", bufs=2, space="PSUM") as tbps:
                for b in range(c.NB):
                    ps = tbps.tile([128, 65], bf16)
                    nc.tensor.transpose(ps[:], hTe[:, b * 128:(b + 1) * 128],
                                        identb_sb[:65, :65])
                    nc.vector.tensor_copy(out=htab_sb[:, b, :65], in_=ps[:])
            nc.sync.dma_start(
                out=tab_sh.rearrange("(b p) w -> p b w", p=128),
                in_=htab_sb[:])
            if sim_mode:
                for _i in range(NCORES):
                    nc.sync.dma_start(
                        out=tab_full[_i * c.NS:(_i + 1) * c.NS, :], in_=tab_sh)
            else:
                nc.gpsimd.collective_compute(
                    "AllGather", mybir.AluOpType.bypass, ins=[tab_sh],
                    outs=[tab_full], replica_groups=RG)

        build_tab(h0Te, htab0_sh, htab0_full)

        # ---------------- edge phase ----------------------------------------
        def edge_phase(tab_full, hTe, Wlb_sb, Wrx_sb, att_sb, W, XW, post_block):
            """XW = xr-matmul rhs width (W + subtract-cols). z/att columns
            are in (c, h)-major order so broadcasts have stride-1 innermost."""
            WP = W + HEADS
            ZG = 2 if W > 64 else c.KQ   # ktiles per z-PSUM tile (bank <= 2KB)
            with tc.tile_pool(name="eg_idx", bufs=2) as idxp, \
                 tc.tile_pool(name="eg_g", bufs=2) as gp, \
                 tc.tile_pool(name="eg_xrb", bufs=2 * CB) as xrbp, \
                 tc.tile_pool(name="eg_oh", bufs=3) as ohp, \
                 tc.tile_pool(name="eg_ohT", bufs=3) as ohTp, \
                 tc.tile_pool(name="eg_sb", bufs=3) as sbp, \
                 tc.tile_pool(name="eg_ywp", bufs=3) as ywpp, \
                 tc.tile_pool(name="eg_nrm", bufs=3) as nrm, \
                 tc.tile_pool(name="eg_xrps", bufs=1, space="PSUM") as xrps, \
                 tc.tile_pool(name="eg_trps", bufs=2, space="PSUM") as trps, \
                 tc.tile_pool(name="eg_zps", bufs=2, space="PSUM") as zps, \
                 tc.tile_pool(name="eg_accps", bufs=2, space="PSUM") as accps:
                for ci, CBc in enumerate(c.CHUNKS):
                    chunk_slots = NQ * CBc * c.CAP
                    s0 = int(c.CHUNK_SLOT0[ci])
                    # chunk idx load
                    idx_t = idxp.tile([128, NQ * CB * c.KQ * 8], i16, name="idx")
                    nc.sync.dma_start(
                        out=idx_t[:, :chunk_slots // 16],
                        in_=srcq_w[:, s0 // 16:(s0 + chunk_slots) // 16])
                    # xr blocks
                    xrbs = []
                    for j in range(CBc):
                        blk = ci * CB + j
                        ps = xrps.tile([128, XW], f32, name="xrps")
                        nc.tensor.matmul(
                            ps[:], hTe[:, blk * 128:(blk + 1) * 128],
                            Wrx_sb[:], start=True, stop=True)
                        xt = xrbp.tile([128, XW], bf16, name="xrb")
                        nc.scalar.activation(xt[:], ps[:], AF.Copy)
                        xrbs.append(xt)
                    # gathers (one per quarter)
                    stripe = CBc * c.KQ * 128
                    g_t = gp.tile([128, NQ, CB * c.KQ * 128], bf16, name="g")
                    for q in range(NQ):
                        gap = g_t[:, q, :stripe]
                        gv = bass.AP(tensor=gap.tensor, offset=gap.offset,
                                     ap=[[gap.ap[0][0], 128], [0, 1], [1, stripe]])
                        nc.gpsimd.dma_gather(
                            gv, tab_full[q * c.QROWS:(q + 1) * c.QROWS, :],
                            idx_t[:, q * stripe // 16:(q + 1) * stripe // 16],
                            stripe, stripe, 128, single_packet=False,
                            transpose=True)
                    # blocks
                    for j in range(CBc):
                        blk = ci * CB + j
                        acc = accps.tile([128, WP], f32, name="acc")
                        for q in range(NQ):
                            cell0 = s0 + (q * CBc + j) * c.CAP
                            col0 = cell0 // 128
                            # one-hot per ktile: (io128 == dstrel_col)
                            oh = ohp.tile([128, c.KQ, 128], bf16, name="oh")
                            for kt in range(c.KQ):
                                nc.vector.tensor_scalar(
                                    oh[:, kt, :], io128_sb[:],
                                    dstrel_sb[:, col0 + kt:col0 + kt + 1], None,
                                    OP.is_equal)
                            # transposed one-hots (PE) -> SBUF via Pool copy
                            tr = trps.tile([128, c.KQ, 128], bf16, name="tr")
                            for kt in range(c.KQ):
                                nc.tensor.transpose(tr[:, kt, :], oh[:, kt, :],
                                                    identb_sb[:])
                            ohT = ohTp.tile([128, c.KQ, 128], bf16, name="ohT")
                            nc.scalar.activation(ohT[:], tr[:], AF.Copy)
                            # z = xl + xr in PSUM, lrelu on ACT
                            zlr = sbp.tile([128, c.KQ, W], bf16, name="zlr")
                            zsb = sbp.tile([128, c.KQ, W], bf16, name="zsb")
                            for p0 in range(0, c.KQ, ZG):
                                pn = min(ZG, c.KQ - p0)
                                psz = zps.tile([128, ZG, W], f32, name="psz")
                                for i in range(pn):
                                    kt = p0 + i
                                    goff = (q * CB + j) * c.CAP + kt * 128
                                    gsl = bass.AP(
                                        tensor=g_t[:].tensor,
                                        offset=g_t[:].offset + goff,
                                        ap=[[g_t[:].ap[0][0], 65], [1, 128]])
                                    nc.tensor.matmul(psz[:, i, :], gsl, Wlb_sb[:],
                                                     start=True, stop=False)
                                    nc.tensor.matmul(psz[:, i, :], ohT[:, kt, :],
                                                     xrbs[j][:, :W],
                                                     start=False, stop=True)
                                nc.scalar.activation(zlr[:, p0:p0 + pn, :],
                                                     psz[:, :pn, :], AF.Prelu,
                                                     alpha=NEG_SLOPE)
                                nc.scalar.activation(zsb[:, p0:p0 + pn, :],
                                                     psz[:, :pn, :], AF.Copy)
                            # per-cell SBUF pipeline (all stride-1, 2-byte)
                            az = sbp.tile([128, c.KQ, W], bf16, name="az")
                            nc.vector.tensor_tensor(
                                out=az[:], in0=zlr[:],
                                in1=bc(att_sb[:, :W], [(0, 128), (None, c.KQ), (1, W)]),
                                op=OP.mult)
                            # fold tree over c (flat halves in (c, h) order)
                            src_t, fw = az, W
                            while fw > 8 * HEADS:
                                nf = fw // 2
                                azf = sbp.tile([128, c.KQ, nf], bf16,
                                               name=f"azf{nf}")
                                nc.vector.tensor_tensor(
                                    out=azf[:], in0=src_t[:, :, :nf],
                                    in1=src_t[:, :, nf:fw], op=OP.add)
                                src_t, fw = azf, nf
                            lg = nrm.tile([128, c.KQ, HEADS], f32, name="lg")
                            sv = src_t[:, :, :fw]
                            nc.vector.tensor_reduce(
                                out=lg[:],
                                in_=bass.AP(tensor=sv.tensor, offset=sv.offset,
                                            ap=sv.ap[:2] + [[1, HEADS],
                                                            [HEADS, fw // HEADS]]),
                                axis=mybir.AxisListType.X, op=OP.add)
                            ywp = ywpp.tile([128, c.KQ, WP], bf16, name="ywp")
                            nc.scalar.activation(ywp[:, :, W:WP], lg[:], AF.Exp)
                            nc.vector.tensor_tensor(
                                out=ywp[:, :, 0:W].rearrange(
                                    "p k (cc h) -> p k cc h", h=HEADS),
                                in0=zsb[:].rearrange("p k (cc h) -> p k cc h",
                                                     h=HEADS),
                                in1=bc(ywp[:, :, W:WP],
                                       [(0, 128), (1, c.KQ), (None, W // HEADS),
                                        (2, HEADS)]),
                                op=OP.mult)
                            for kt in range(c.KQ):
                                nc.tensor.matmul(
                                    acc[:], oh[:, kt, :], ywp[:, kt, :],
                                    start=(q == 0 and kt == 0),
                                    stop=(q == NQ - 1 and kt == c.KQ - 1))
                        post_block(blk, acc, xrbs[j], nrm, xrps)

        # ---- L0 post: normalize -> htab_sb + h1Te
        def l0_post(blk, acc, xrb, nrm, auxps):
            W = HID
            d4 = nrm.tile([128, HEADS], f32, name="d4")
            nc.vector.tensor_scalar_max(d4[:], acc[:, W:W + HEADS], 1e-30)
            r4 = nrm.tile([128, HEADS], f32, name="r4")
            nc.vector.reciprocal(r4[:], d4[:])
            vm = nrm.tile([128, W], f32, name="vm")
            nc.vector.tensor_tensor(
                out=vm[:].rearrange("p (cc h) -> p cc h", h=HEADS),
                in0=acc[:, 0:W].rearrange("p (cc h) -> p cc h", h=HEADS),
                in1=bc(r4[:], [(0, 128), (None, W // HEADS), (1, HEADS)]),
                op=OP.mult)
            v2 = nrm.tile([128, W], f32, name="v2")
            nc.vector.tensor_tensor(out=v2[:], in0=vm[:],
                                    in1=bc(A0_sb[:, :W], [(0, 128), (1, W)]),
                                    op=OP.mult)
            v3 = nrm.tile([128, W], f32, name="v3")
            nc.vector.tensor_tensor(out=v3[:], in0=v2[:], in1=xrb[:, W:2 * W],
                                    op=OP.subtract)
            nc.vector.tensor_scalar_max(htab_sb[:, blk, :W], v3[:], 0.0)
            tr = auxps.tile([64, 128], bf16, name="trh")
            nc.tensor.transpose(tr[:], htab_sb[:, blk, :W], identb_sb[:])
            nc.vector.tensor_copy(out=h1Te[:64, blk * 128:(blk + 1) * 128],
                                  in_=tr[:])

        edge_phase(htab0_full, h0Te, Wl0b_sb, Wr0x_sb, att0_sb, HID, 2 * HID,
                   l0_post)

        # ---------------- h1 table + AllGather ------------------------------
        build_tab(h1Te, htab1_sh, htab1_full)

        # ---------------- L1 edge phase + pooling ---------------------------
        with tc.tile_pool(name="poolps", bufs=1, space="PSUM") as poolps, \
             tc.tile_pool(name="poolsb", bufs=2) as poolsb:
            pooled_ps = poolps.tile([c.G, HID + 1], f32)

            def l1_post(blk, acc, xrb, nrm, auxps):
                W = 256
                d4 = nrm.tile([128, HEADS], f32, name="d4")
                nc.vector.tensor_scalar_max(d4[:], acc[:, W:W + HEADS], 1e-30)
                r4 = nrm.tile([128, HEADS], f32, name="r4")
                nc.vector.reciprocal(r4[:], d4[:])
                vm = nrm.tile([128, W], f32, name="vm1")
                nc.vector.tensor_tensor(
                    out=vm[:].rearrange("p (cc h) -> p cc h", h=HEADS),
                    in0=acc[:, 0:W].rearrange("p (cc h) -> p cc h", h=HEADS),
                    in1=bc(r4[:], [(0, 128), (None, HID), (1, HEADS)]),
                    op=OP.mult)
                hm = nrm.tile([128, HID], f32, name="hm")
                nc.vector.tensor_reduce(
                    out=hm[:], in_=vm[:].rearrange("p (cc h) -> p cc h", h=HEADS),
                    axis=mybir.AxisListType.X, op=OP.add)
                va = nrm.tile([128, HID], f32, name="va")
                nc.vector.tensor_tensor(out=va[:], in0=hm[:],
                                        in1=bc(A1q_sb[:, :HID], [(0, 128), (1, HID)]),
                                        op=OP.mult)
                vb = nrm.tile([128, HID], f32, name="vb")
                nc.vector.tensor_tensor(out=vb[:], in0=va[:],
                                        in1=xrb[:, W:W + HID], op=OP.subtract)
                hpool = poolsb.tile([128, HID + 1], bf16, name="hpool")
                nc.vector.memset(hpool[:, HID:HID + 1], 1.0)
                nc.vector.tensor_scalar_max(hpool[:, :HID], vb[:], 0.0)
                og = poolsb.tile([128, c.G], bf16, name="og")
                nc.vector.tensor_tensor(
                    out=og[:],
                    in0=bc(batchloc_sb[:, blk:blk + 1], [(0, 128), (None, c.G)]),
                    in1=bc(ioG_sb[:], [(0, 128), (1, c.G)]),
                    op=OP.is_equal)
                nc.tensor.matmul(pooled_ps[:], og[:], hpool[:],
                                 start=(blk == 0), stop=(blk == c.NB - 1))

            edge_phase(htab1_full, h1Te, Wl1b_sb, Wr1x_sb, att1_sb, 256, 320,
                       l1_post)

            pool_sb = poolsb.tile([c.G, HID + 1], f32, name="poolsb")
            nc.vector.tensor_copy(out=pool_sb[:], in_=pooled_ps[:])
            nc.sync.dma_start(out=pool_in[:, :], in_=pool_sb[:])

        if sim_mode:
            nc.sync.dma_start(out=pool_out[:, :], in_=pool_in[:, :])
        else:
            nc.gpsimd.collective_compute(
                "AllReduce", mybir.AluOpType.add, ins=[pool_in],
                outs=[pool_out], replica_groups=RG)

        # ---------------- classifier ---------------------------------------
        with tc.tile_pool(name="cls", bufs=1) as cls, \
             tc.tile_pool(name="clsps", bufs=2, space="PSUM") as clsps:
            pall = cls.tile([c.G, HID + 1], f32)
            nc.sync.dma_start(out=pall[:], in_=pool_out[:, :])
            cnt = cls.tile([c.G, 1], f32)
            nc.vector.tensor_scalar_max(cnt[:], pall[:, HID:HID + 1], 1.0)
            rcnt = cls.tile([c.G, 1], f32)
            nc.vector.reciprocal(rcnt[:], cnt[:])
            pm = cls.tile([c.G, HID], f32)
            nc.vector.tensor_scalar_mul(pm[:], pall[:, :HID], rcnt[:])
            pmT_ps = clsps.tile([HID, c.G], f32)
            nc.tensor.transpose(pmT_ps[:], pm[:], ident32_sb[:c.G, :c.G])
            pmT = cls.tile([HID, c.G], f32)
            nc.vector.tensor_copy(out=pmT[:], in_=pmT_ps[:])
            z1_ps = clsps.tile([32, c.G], f32)
            nc.tensor.matmul(z1_ps[:], Wc1_sb[:], pmT[:], start=True, stop=True)
            z1 = cls.tile([32, c.G], f32)
            nc.scalar.activation(z1[:], z1_ps[:], AF.Relu, bias=bc1_sb[:])
            o_ps = clsps.tile([2, c.G], f32)
            nc.tensor.matmul(o_ps[:], Wc2_sb[:], z1[:], start=True, stop=True)
            o_sb = cls.tile([2, c.G], f32)
            nc.scalar.activation(o_sb[:], o_ps[:], AF.Identity, bias=bc2_sb[:])
            nc.sync.dma_start(out=out_t[:, :], in_=o_sb[:])

    nc.compile()
    return nc


# ---------------------------------------------------------------- driver ----
_BUILT = {}
LAST_RESULTS = None


def _get_program(cfg):
    key = (cfg.N, cfg.E, cfg.G, cfg.KQ)
    if key not in _BUILT:
        _BUILT[key] = build_program(cfg)
    return _BUILT[key]


def kernel(**inputs):
    from concourse import bass_utils

    x = np.asarray(inputs["x"])
    edge_index = np.asarray(inputs["edge_index"])
    N = x.shape[0]
    E = edge_index.shape[1]
    G = 64
    KQ = compute_kq(inputs)
    cfg = Cfg(N, E, G, KQ)
    per_core = host_prep(inputs, cfg)
    nc = _get_program(cfg)
    in_maps = [{k: np.ascontiguousarray(v) for k, v in m.items()} for m in per_core]
    res = bass_utils.run_bass_kernel_spmd(nc, in_maps, core_ids=list(range(NCORES)))
    global LAST_RESULTS
    LAST_RESULTS = res
    out = res.results[0]["out_t"]  # [2, G]
    return np.ascontiguousarray(out.T.astype(np.float32))



# revision 8
# speedup vs baseline: 1.4995x; 1.4995x over previous
"""Trainium2 Bass kernel for a 2-layer GATv2 GNN (nn_ComponentGNN), v2.

Strategy vs v1: gather narrow bf16 node-feature rows (256B) with
dma_gather(transpose=True) and apply the GAT linear transforms on-chip with
bf16 matmuls; per-edge xr comes from a one-hot-transpose matmul against a
per-block xr tile (no DRAM gather); messages use the z-form trick
(scatter z*w, correct with xr*den after normalize); leaky-relu runs on the
ACT engine (Prelu alpha), the per-head logit reduce on the Pool engine.

Sharding: nodes (and their in-edges grouped by destination block) across 8
cores; weights replicated; bf16 node tables AllGather'd between layers;
per-graph pooled sums AllReduced.
"""
import math
import sys

import numpy as np

sys.path.insert(0, "/opt/trn_rl_repo")

NEG_SLOPE = 0.2
EPS = 1e-5
HEADS = 4
HID = 64
F_IN = 256
NCORES = 8
NQ = 4          # src-table quarters (dma_gather idx is int16)
CB = 4          # dst blocks per chunk (gather granularity)


def as_bf16(x):
    import jax.numpy as jnp
    return np.asarray(jnp.asarray(np.asarray(x, np.float32), jnp.bfloat16))


# ---------------------------------------------------------------- config ----
class Cfg:
    def __init__(self, N, E, G, KQ):
        self.N, self.E, self.G = N, E, G
        assert N % NCORES == 0
        self.NS_REAL = N // NCORES
        self.NS = ((self.NS_REAL + 127) // 128) * 128
        self.NB = self.NS // 128
        self.NTAB = self.NS * NCORES
        assert self.NTAB % NQ == 0
        self.QROWS = self.NTAB // NQ
        assert self.QROWS <= 32768
        self.KQ = KQ
        self.CAP = KQ * 128
        # chunks of CB dst blocks
        self.CHUNKS = []
        b = 0
        while b < self.NB:
            self.CHUNKS.append(min(CB, self.NB - b))
            b += CB
        # slot base of each chunk
        self.CHUNK_SLOT0 = np.concatenate(
            [[0], np.cumsum([NQ * c * self.CAP for c in self.CHUNKS])]).astype(np.int64)
        self.S = int(self.CHUNK_SLOT0[-1])
        assert self.S == self.NB * NQ * self.CAP


def compute_kq(inputs, cfg_like=None):
    edge_index = np.asarray(inputs["edge_index"])
    N = int(np.asarray(inputs["x"]).shape[0])
    NS_REAL = N // NCORES
    NS = ((NS_REAL + 127) // 128) * 128
    NB = NS // 128
    QROWS = NS * NCORES // NQ
    src = edge_index[0].astype(np.int64)
    dst = edge_index[1].astype(np.int64)
    core = dst // NS_REAL
    dstl = dst % NS_REAL
    blk = dstl // 128
    srow = (src // NS_REAL) * NS + (src % NS_REAL)
    q = srow // QROWS
    keys = (core * NQ + q) * NB + blk
    cnt = np.bincount(keys, minlength=NCORES * NQ * NB)
    return int(math.ceil(cnt.max() / 128))


def host_prep(inputs, cfg):
    c = cfg
    edge_index = np.asarray(inputs["edge_index"])
    batch = np.asarray(inputs["batch"])
    src = edge_index[0].astype(np.int64)
    dst = edge_index[1].astype(np.int64)

    core = dst // c.NS_REAL
    dstl = dst % c.NS_REAL
    blk = dstl // 128
    srow = (src // c.NS_REAL) * c.NS + (src % c.NS_REAL)
    q = srow // c.QROWS
    srcq = srow % c.QROWS

    ci = blk // CB                      # chunk index
    j = blk % CB                        # block within chunk
    cbc = np.minimum(CB, c.NB - ci * CB)
    slot_base = c.CHUNK_SLOT0[ci] + (q * cbc + j) * c.CAP

    order = np.lexsort((slot_base, core))
    so_core = core[order]
    so_base = slot_base[order]
    cell_key = so_core.astype(np.int64) * c.S + so_base
    chg = np.empty(len(cell_key), dtype=bool)
    chg[0] = True
    chg[1:] = cell_key[1:] != cell_key[:-1]
    seg_start = np.maximum.accumulate(np.where(chg, np.arange(len(cell_key)), 0))
    slot = so_base + (np.arange(len(cell_key)) - seg_start)

    srcq_idx = np.zeros((NCORES, c.S), np.int16)
    dstrel = np.full((NCORES, c.S), -1.0, np.float32)
    srcq_idx[so_core, slot] = srcq[order].astype(np.int16)
    dstrel[so_core, slot] = (dstl[order] - blk[order] * 128).astype(np.float32)

    def wrap16(a):  # [S] int16 -> [128, S//16]
        w = a.reshape(-1, 16).T
        return np.tile(w, (8, 1)).copy()

    srcq_w = np.stack([wrap16(srcq_idx[i]) for i in range(NCORES)])
    dstrel_pm = dstrel.reshape(NCORES, c.S // 128, 128).transpose(0, 2, 1)

    batchloc = np.full((NCORES, c.NS), -1.0, np.float32)
    for i in range(NCORES):
        batchloc[i, :c.NS_REAL] = batch[i * c.NS_REAL:(i + 1) * c.NS_REAL]
    batchloc_pm = batchloc.reshape(NCORES, c.NB, 128).transpose(0, 2, 1)

    x = np.asarray(inputs["x"], np.float32)
    xT = np.zeros((NCORES, F_IN, c.NS), np.float32)
    for i in range(NCORES):
        xT[i, :, :c.NS_REAL] = x[i * c.NS_REAL:(i + 1) * c.NS_REAL].T

    f64 = lambda k: np.asarray(inputs[k], np.float64)

    def fold(bias, g, be, m, v):
        a = f64(g) / np.sqrt(f64(v) + EPS)
        B = a * (f64(bias) - f64(m)) + f64(be)
        return a, B

    A0, B0 = fold("bias0", "g0", "be0", "m0", "v0")
    A1, B1 = fold("bias1", "g1", "be1", "m1", "v1")
    A1q = A1 * 0.25

    # (c, h)-major column permutation: new col c*H + h <- old col h*ch + c.
    def chperm(ch):
        p = np.empty(HEADS * ch, np.int64)
        for h in range(HEADS):
            for cc in range(ch):
                p[cc * HEADS + h] = h * ch + cc
        return p

    P0 = chperm(16)   # L0: 64 z-cols / output dims
    P1 = chperm(64)   # L1: 256 z-cols

    # z-path weights with bias row appended ([65, W])
    def wb(Wk, bk):
        return np.concatenate([f64(Wk), f64(bk).reshape(1, -1)], axis=0)

    # --- L0 (everything in its z/output space permuted by P0)
    Wl0b = wb("Wl0", "bl0")[:, P0]                # [65, 64]
    Wr0b = wb("Wr0", "br0")[:, P0]                # [65, 64]
    A0p, B0p = A0[P0], B0[P0]
    sub0 = Wr0b * A0p[None, :]
    sub0[64, :] -= B0p
    Wr0x = np.concatenate([Wr0b, sub0], axis=1)   # [65, 128]
    att0 = f64("att0").reshape(-1)[P0].reshape(1, -1)   # [1, 64]

    # --- L1 (rows are h1 dims -> permuted by P0; z-cols permuted by P1)
    def rowperm(W65):
        out = W65.copy()
        out[:64, :] = W65[:64, :][P0, :]
        return out

    Wl1b = rowperm(wb("Wl1", "bl1"))[:, P1]       # [65, 256]
    Wr1b = rowperm(wb("Wr1", "br1"))              # [65, 256] natural cols
    Wr1h = Wr1b.reshape(65, HEADS, HID).sum(axis=1)   # sum over heads [65, 64]
    sub1 = Wr1h * A1q[None, :]
    sub1[64, :] -= B1
    Wr1x = np.concatenate([Wr1b[:, P1], sub1], axis=1)  # [65, 320]
    att1 = f64("att1").reshape(-1)[P1].reshape(1, -1)   # [1, 256]

    rep = dict(
        W_in=as_bf16(inputs["W_in"]),
        b_in_c=np.asarray(inputs["b_in"], np.float32).reshape(-1, 1),
        Wl0b=as_bf16(Wl0b), Wr0x=as_bf16(Wr0x),
        Wl1b=as_bf16(Wl1b), Wr1x=as_bf16(Wr1x),
        att0_r=as_bf16(att0), att1_r=as_bf16(att1),
        A0_r=as_bf16(A0p.reshape(1, -1)), A1q_r=as_bf16(A1q.reshape(1, -1)),
        io128_r=as_bf16(np.arange(128, dtype=np.float32).reshape(1, -1)),
        iotaG_r=as_bf16(np.arange(cfg.G, dtype=np.float32).reshape(1, -1)),
        identb=as_bf16(np.eye(128, dtype=np.float32)),
        ident32=np.eye(128, dtype=np.float32),
        Wc1=np.asarray(inputs["Wc1"], np.float32),
        bc1_c=np.asarray(inputs["bc1"], np.float32).reshape(-1, 1),
        Wc2=np.asarray(inputs["Wc2"], np.float32),
        bc2_c=np.asarray(inputs["bc2"], np.float32).reshape(-1, 1),
    )
    per_core = [dict(
        xT=as_bf16(xT[i]), srcq_w=srcq_w[i],
        dstrel=np.ascontiguousarray(dstrel_pm[i], np.float32),
        batchloc=as_bf16(batchloc_pm[i]),
        **rep,
    ) for i in range(NCORES)]
    return per_core


# ---------------------------------------------------------------- device ----
def build_program(cfg, reps=1, sim_mode=False):
    from contextlib import ExitStack
    import concourse.bass as bass
    import concourse.tile as tile
    from concourse import bacc, mybir

    c = cfg
    f32 = mybir.dt.float32
    bf16 = mybir.dt.bfloat16
    i16 = mybir.dt.int16
    AF = mybir.ActivationFunctionType
    OP = mybir.AluOpType

    nc = bacc.Bacc("TRN2", target_bir_lowering=False, debug=False,
                   num_devices=NCORES)

    def din(name, shape, dt=f32):
        return nc.dram_tensor(name, list(shape), dt, kind="ExternalInput").ap()

    xT = din("xT", [F_IN, c.NS], bf16)
    srcq_w = din("srcq_w", [128, c.S // 16], i16)
    dstrel_d = din("dstrel", [128, c.S // 128])
    batchloc_d = din("batchloc", [128, c.NB], bf16)
    W_in = din("W_in", [F_IN, HID], bf16)
    b_in_c = din("b_in_c", [HID, 1])
    Wl0b_d = din("Wl0b", [65, HID], bf16)
    Wr0x_d = din("Wr0x", [65, 2 * HID], bf16)
    Wl1b_d = din("Wl1b", [65, 256], bf16)
    Wr1x_d = din("Wr1x", [65, 320], bf16)
    att0_r = din("att0_r", [1, HID], bf16)
    att1_r = din("att1_r", [1, 256], bf16)
    A0_r = din("A0_r", [1, HID], bf16)
    A1q_r = din("A1q_r", [1, HID], bf16)
    io128_r = din("io128_r", [1, 128], bf16)
    iotaG_r = din("iotaG_r", [1, c.G], bf16)
    identb_d = din("identb", [128, 128], bf16)
    ident32_d = din("ident32", [128, 128])
    Wc1 = din("Wc1", [HID, 32]); bc1_c = din("bc1_c", [32, 1])
    Wc2 = din("Wc2", [32, 2]); bc2_c = din("bc2_c", [2, 1])

    out_t = nc.dram_tensor("out_t", [2, c.G], f32, kind="ExternalOutput").ap()

    def dram(name, shape, dt=f32, shared=False):
        return nc.dram_tensor(name, list(shape), dt, kind="Internal",
                              addr_space="Shared" if shared else "Local").ap()

    htab0_sh = dram("htab0_sh", [c.NS, 128], bf16)
    htab0_full = dram("htab0_full", [c.NTAB, 128], bf16, shared=True)
    htab1_sh = dram("htab1_sh", [c.NS, 128], bf16)
    htab1_full = dram("htab1_full", [c.NTAB, 128], bf16, shared=True)
    pool_in = dram("pool_in", [c.G, HID + 1])
    pool_out = dram("pool_out", [c.G, HID + 1], shared=True)

    RG = [list(range(NCORES))]

    def bc(ap, dims):
        pat = []
        for ax, size in dims:
            if ax is None:
                pat.append([0, size])
            else:
                st, sz = ap.ap[ax]
                assert sz == size, (ap.ap, ax, size)
                pat.append([st, size])
        return bass.AP(tensor=ap.tensor, offset=ap.offset, ap=pat)

    with tile.TileContext(nc) as tc, ExitStack() as ctx:
        singles = ctx.enter_context(tc.tile_pool(name="singles", bufs=1))
        _cn = [0]

        def load_const(ap_d, shape, dt=f32):
            _cn[0] += 1
            t = singles.tile(list(shape), dt, name=f"c{_cn[0]}")
            nc.sync.dma_start(out=t[:], in_=ap_d)
            return t

        def load_row128(ap_d, width, dt=bf16):
            _cn[0] += 1
            t = singles.tile([128, width], dt, name=f"c{_cn[0]}")
            src = bass.AP(tensor=ap_d.tensor, offset=ap_d.offset,
                          ap=[[0, 128], [1, width]])
            nc.sync.dma_start(out=t[:], in_=src)
            return t

        W_in_h = []
        for kh in range(F_IN // 128):
            t = singles.tile([128, HID], bf16, name=f"Win{kh}")
            nc.sync.dma_start(out=t[:], in_=W_in[kh * 128:(kh + 1) * 128, :])
            W_in_h.append(t)
        b_in_sb = load_const(b_in_c, [HID, 1])
        Wl0b_sb = load_const(Wl0b_d, [65, HID], bf16)
        Wr0x_sb = load_const(Wr0x_d, [65, 2 * HID], bf16)
        Wl1b_sb = load_const(Wl1b_d, [65, 256], bf16)
        Wr1x_sb = load_const(Wr1x_d, [65, 320], bf16)
        att0_sb = load_row128(att0_r, HID)
        att1_sb = load_row128(att1_r, 256)
        A0_sb = load_row128(A0_r, HID)
        A1q_sb = load_row128(A1q_r, HID)
        io128_sb = load_row128(io128_r, 128)
        ioG_sb = load_row128(iotaG_r, c.G)
        identb_sb = load_const(identb_d, [128, 128], bf16)
        ident32_sb = load_const(ident32_d, [128, 128])
        Wc1_sb = load_const(Wc1, [HID, 32]); bc1_sb = load_const(bc1_c, [32, 1])
        Wc2_sb = load_const(Wc2, [32, 2]); bc2_sb = load_const(bc2_c, [2, 1])
        batchloc_sb = load_const(batchloc_d, [128, c.NB], bf16)
        dstrel_sb = load_const(dstrel_d, [128, c.S // 128])

        # resident transposed node features [65, NS] bf16 (row 64 = ones)
        h0Te = singles.tile([65, c.NS], bf16, name="h0Te")
        h1Te = singles.tile([65, c.NS], bf16, name="h1Te")
        htab_sb = singles.tile([128, c.NB, 128], bf16, name="htab_sb")
        nc.gpsimd.memset(h0Te[64:65, :], 1.0)
        nc.gpsimd.memset(h1Te[64:65, :], 1.0)
        nc.gpsimd.memset(htab_sb[:, :, 64:65], 1.0)
        nc.gpsimd.memset(htab_sb[:, :, 65:128], 0.0)

        # ---------------- P1: h0Te = relu(W_in^T x + b), bf16 ---------------
        with tc.tile_pool(name="p1", bufs=3) as p1, \
             tc.tile_pool(name="p1ps", bufs=2, space="PSUM") as p1ps:
            n0 = 0
            while n0 < c.NS:
                nw = min(512, c.NS - n0)
                ps = p1ps.tile([HID, 512], f32)
                for kh in range(F_IN // 128):
                    xt_t = p1.tile([128, 512], bf16, name="xt")
                    nc.sync.dma_start(out=xt_t[:, :nw],
                                      in_=xT[kh * 128:(kh + 1) * 128, n0:n0 + nw])
                    nc.tensor.matmul(ps[:, :nw], W_in_h[kh][:], xt_t[:, :nw],
                                     start=(kh == 0), stop=(kh == F_IN // 128 - 1))
                nc.scalar.activation(h0Te[:64, n0:n0 + nw], ps[:, :nw], AF.Relu,
                                     bias=b_in_sb[:], scale=1.0)
                n0 += nw

        # ---------------- h0 table shard (node-major bf16) + AllGather ------
        def build_tab(hTe, tab_sh, tab_full, transpose=True):
            if transpose:
                with tc.tile_pool(name="tb_ps", bufs=2, space="PSUM") as tbps:
                    for b in range(c.NB):
                        ps = tbps.tile([128, 65], bf16)
                        nc.tensor.transpose(ps[:], hTe[:, b * 128:(b + 1) * 128],
                                            identb_sb[:65, :65])
                        nc.vector.tensor_copy(out=htab_sb[:, b, :65], in_=ps[:])
            nc.sync.dma_start(
                out=tab_sh.rearrange("(b p) w -> p b w", p=128),
                in_=htab_sb[:])
            if sim_mode:
                for _i in range(NCORES):
                    nc.sync.dma_start(
                        out=tab_full[_i * c.NS:(_i + 1) * c.NS, :], in_=tab_sh)
            else:
                nc.gpsimd.collective_compute(
                    "AllGather", mybir.AluOpType.bypass, ins=[tab_sh],
                    outs=[tab_full], replica_groups=RG)

        build_tab(h0Te, htab0_sh, htab0_full)

        # ---------------- edge phase ----------------------------------------
        def edge_phase(tab_full, hTe, Wlb_sb, Wrx_sb, att_sb, W, XW, post_block):
            """XW = xr-matmul rhs width (W + subtract-cols). z/att columns
            are in (c, h)-major order so broadcasts have stride-1 innermost."""
            WP = W + HEADS
            ZG = 2 if W > 64 else c.KQ   # ktiles per z-PSUM tile (bank <= 2KB)
            with tc.tile_pool(name="eg_idx", bufs=2) as idxp, \
                 tc.tile_pool(name="eg_g", bufs=2) as gp, \
                 tc.tile_pool(name="eg_xrb", bufs=2 * CB) as xrbp, \
                 tc.tile_pool(name="eg_oh", bufs=4) as ohp, \
                 tc.tile_pool(name="eg_ohT", bufs=4) as ohTp, \
                 tc.tile_pool(name="eg_sb", bufs=3) as sbp, \
                 tc.tile_pool(name="eg_ywp", bufs=3) as ywpp, \
                 tc.tile_pool(name="eg_nrm", bufs=4) as nrm, \
                 tc.tile_pool(name="eg_xrps", bufs=1, space="PSUM") as xrps, \
                 tc.tile_pool(name="eg_trps", bufs=2, space="PSUM") as trps, \
                 tc.tile_pool(name="eg_zps", bufs=2, space="PSUM") as zps, \
                 tc.tile_pool(name="eg_accps", bufs=2, space="PSUM") as accps:
                for ci, CBc in enumerate(c.CHUNKS):
                    chunk_slots = NQ * CBc * c.CAP
                    s0 = int(c.CHUNK_SLOT0[ci])
                    # chunk idx load
                    idx_t = idxp.tile([128, NQ * CB * c.KQ * 8], i16, name="idx")
                    nc.sync.dma_start(
                        out=idx_t[:, :chunk_slots // 16],
                        in_=srcq_w[:, s0 // 16:(s0 + chunk_slots) // 16])
                    # xr blocks
                    xrbs = []
                    for j in range(CBc):
                        blk = ci * CB + j
                        ps = xrps.tile([128, XW], f32, name="xrps")
                        nc.tensor.matmul(
                            ps[:], hTe[:, blk * 128:(blk + 1) * 128],
                            Wrx_sb[:], start=True, stop=True)
                        xt = xrbp.tile([128, XW], bf16, name="xrb")
                        nc.scalar.activation(xt[:], ps[:], AF.Copy)
                        xrbs.append(xt)
                    # gathers (one per quarter)
                    stripe = CBc * c.KQ * 128
                    g_t = gp.tile([128, NQ, CB * c.KQ * 128], bf16, name="g")
                    for q in range(NQ):
                        gap = g_t[:, q, :stripe]
                        gv = bass.AP(tensor=gap.tensor, offset=gap.offset,
                                     ap=[[gap.ap[0][0], 128], [0, 1], [1, stripe]])
                        nc.gpsimd.dma_gather(
                            gv, tab_full[q * c.QROWS:(q + 1) * c.QROWS, :],
                            idx_t[:, q * stripe // 16:(q + 1) * stripe // 16],
                            stripe, stripe, 128, single_packet=False,
                            transpose=True)
                    # blocks
                    for j in range(CBc):
                        blk = ci * CB + j
                        acc = accps.tile([128, WP], f32, name="acc")
                        for q in range(NQ):
                            cell0 = s0 + (q * CBc + j) * c.CAP
                            col0 = cell0 // 128
                            # one-hot per ktile: (io128 == dstrel_col)
                            oh = ohp.tile([128, c.KQ, 128], bf16, name="oh")
                            for kt in range(c.KQ):
                                nc.vector.tensor_scalar(
                                    oh[:, kt, :], io128_sb[:],
                                    dstrel_sb[:, col0 + kt:col0 + kt + 1], None,
                                    OP.is_equal)
                            # transposed one-hots (PE) -> SBUF via Pool copy
                            tr = trps.tile([128, c.KQ, 128], bf16, name="tr")
                            for kt in range(c.KQ):
                                nc.tensor.transpose(tr[:, kt, :], oh[:, kt, :],
                                                    identb_sb[:])
                            ohT = ohTp.tile([128, c.KQ, 128], bf16, name="ohT")
                            if W > 64 and q >= 2:
                                nc.vector.tensor_copy(out=ohT[:], in_=tr[:])
                            else:
                                nc.scalar.activation(ohT[:], tr[:], AF.Copy)
                            # z = xl + xr in PSUM, lrelu on ACT
                            zlr = sbp.tile([128, c.KQ, W], bf16, name="zlr")
                            pszs = []
                            for p0 in range(0, c.KQ, ZG):
                                pn = min(ZG, c.KQ - p0)
                                psz = zps.tile([128, ZG, W], f32, name="psz")
                                pszs.append((p0, pn, psz))
                                for i in range(pn):
                                    kt = p0 + i
                                    goff = (q * CB + j) * c.CAP + kt * 128
                                    gsl = bass.AP(
                                        tensor=g_t[:].tensor,
                                        offset=g_t[:].offset + goff,
                                        ap=[[g_t[:].ap[0][0], 65], [1, 128]])
                                    nc.tensor.matmul(psz[:, i, :], gsl, Wlb_sb[:],
                                                     start=True, stop=False)
                                    nc.tensor.matmul(psz[:, i, :], ohT[:, kt, :],
                                                     xrbs[j][:, :W],
                                                     start=False, stop=True)
                                nc.scalar.activation(zlr[:, p0:p0 + pn, :],
                                                     psz[:, :pn, :], AF.Prelu,
                                                     alpha=NEG_SLOPE)
                            if W > 64:
                                # L1: recover z from zlr (inverse prelu) on ACT
                                zsb = sbp.tile([128, c.KQ, W], bf16, name="zsb")
                                nc.scalar.activation(zsb[:], zlr[:], AF.Prelu,
                                                     alpha=1.0 / NEG_SLOPE)
                            # per-cell SBUF pipeline (all stride-1, 2-byte)
                            az = sbp.tile([128, c.KQ, W], bf16, name="az")
                            nc.vector.tensor_tensor(
                                out=az[:], in0=zlr[:],
                                in1=bc(att_sb[:, :W], [(0, 128), (None, c.KQ), (1, W)]),
                                op=OP.mult)
                            # fold tree over c (flat halves in (c, h) order)
                            src_t, fw = az, W
                            while fw > 8 * HEADS:
                                nf = fw // 2
                                azf = sbp.tile([128, c.KQ, nf], bf16,
                                               name=f"azf{nf}")
                                nc.vector.tensor_tensor(
                                    out=azf[:], in0=src_t[:, :, :nf],
                                    in1=src_t[:, :, nf:fw], op=OP.add)
                                src_t, fw = azf, nf
                            lg = nrm.tile([128, c.KQ, HEADS], f32, name="lg")
                            sv = src_t[:, :, :fw]
                            nc.vector.tensor_reduce(
                                out=lg[:],
                                in_=bass.AP(tensor=sv.tensor, offset=sv.offset,
                                            ap=sv.ap[:2] + [[1, HEADS],
                                                            [HEADS, fw // HEADS]]),
                                axis=mybir.AxisListType.X, op=OP.add)
                            ywp = ywpp.tile([128, c.KQ, WP], bf16, name="ywp")
                            nc.scalar.activation(ywp[:, :, W:WP], lg[:], AF.Exp)
                            if W > 64:
                                nc.vector.tensor_tensor(
                                    out=ywp[:, :, 0:W].rearrange(
                                        "p k (cc h) -> p k cc h", h=HEADS),
                                    in0=zsb[:].rearrange("p k (cc h) -> p k cc h",
                                                         h=HEADS),
                                    in1=bc(ywp[:, :, W:WP],
                                           [(0, 128), (1, c.KQ), (None, W // HEADS),
                                            (2, HEADS)]),
                                    op=OP.mult)
                            else:
                                # L0: multiply raw z straight out of PSUM
                                for p0, pn, psz in pszs:
                                    nc.vector.tensor_tensor(
                                        out=ywp[:, p0:p0 + pn, 0:W].rearrange(
                                            "p k (cc h) -> p k cc h", h=HEADS),
                                        in0=psz[:, :pn, :].rearrange(
                                            "p k (cc h) -> p k cc h", h=HEADS),
                                        in1=bc(ywp[:, p0:p0 + pn, W:WP],
                                               [(0, 128), (1, pn), (None, W // HEADS),
                                                (2, HEADS)]),
                                        op=OP.mult)
                            for kt in range(c.KQ):
                                nc.tensor.matmul(
                                    acc[:], oh[:, kt, :], ywp[:, kt, :],
                                    start=(q == 0 and kt == 0),
                                    stop=(q == NQ - 1 and kt == c.KQ - 1))
                        post_block(blk, acc, xrbs[j], nrm, xrps)

        # ---- L0 post: normalize -> htab_sb + h1Te
        def l0_post(blk, acc, xrb, nrm, auxps):
            W = HID
            d4 = nrm.tile([128, HEADS], f32, name="d4")
            nc.vector.tensor_scalar_max(d4[:], acc[:, W:W + HEADS], 1e-30)
            r4 = nrm.tile([128, HEADS], f32, name="r4")
            nc.vector.reciprocal(r4[:], d4[:])
            vm = nrm.tile([128, W], f32, name="vm")
            nc.vector.tensor_tensor(
                out=vm[:].rearrange("p (cc h) -> p cc h", h=HEADS),
                in0=acc[:, 0:W].rearrange("p (cc h) -> p cc h", h=HEADS),
                in1=bc(r4[:], [(0, 128), (None, W // HEADS), (1, HEADS)]),
                op=OP.mult)
            v2 = nrm.tile([128, W], f32, name="v2")
            nc.vector.tensor_tensor(out=v2[:], in0=vm[:],
                                    in1=bc(A0_sb[:, :W], [(0, 128), (1, W)]),
                                    op=OP.mult)
            v3 = nrm.tile([128, W], f32, name="v3")
            nc.vector.tensor_tensor(out=v3[:], in0=v2[:], in1=xrb[:, W:2 * W],
                                    op=OP.subtract)
            nc.vector.tensor_scalar_max(htab_sb[:, blk, :W], v3[:], 0.0)
            tr = auxps.tile([64, 128], bf16, name="trh")
            nc.tensor.transpose(tr[:], htab_sb[:, blk, :W], identb_sb[:])
            nc.vector.tensor_copy(out=h1Te[:64, blk * 128:(blk + 1) * 128],
                                  in_=tr[:])

        edge_phase(htab0_full, h0Te, Wl0b_sb, Wr0x_sb, att0_sb, HID, 2 * HID,
                   l0_post)

        # ---------------- h1 table + AllGather ------------------------------
        # htab_sb already holds h1 rows (written by l0_post); skip re-transpose
        build_tab(h1Te, htab1_sh, htab1_full, transpose=False)

        # ---------------- L1 edge phase + pooling ---------------------------
        with tc.tile_pool(name="poolps", bufs=1, space="PSUM") as poolps, \
             tc.tile_pool(name="poolsb", bufs=2) as poolsb:
            pooled_ps = poolps.tile([c.G, HID + 1], f32)

            def l1_post(blk, acc, xrb, nrm, auxps):
                W = 256
                d4 = nrm.tile([128, HEADS], f32, name="d4")
                nc.vector.tensor_scalar_max(d4[:], acc[:, W:W + HEADS], 1e-30)
                r4 = nrm.tile([128, HEADS], f32, name="r4")
                nc.vector.reciprocal(r4[:], d4[:])
                vm = nrm.tile([128, W], f32, name="vm1")
                nc.vector.tensor_tensor(
                    out=vm[:].rearrange("p (cc h) -> p cc h", h=HEADS),
                    in0=acc[:, 0:W].rearrange("p (cc h) -> p cc h", h=HEADS),
                    in1=bc(r4[:], [(0, 128), (None, HID), (1, HEADS)]),
                    op=OP.mult)
                hm = nrm.tile([128, HID], f32, name="hm")
                nc.vector.tensor_reduce(
                    out=hm[:], in_=vm[:].rearrange("p (cc h) -> p cc h", h=HEADS),
                    axis=mybir.AxisListType.X, op=OP.add)
                va = nrm.tile([128, HID], f32, name="va")
                nc.vector.tensor_tensor(out=va[:], in0=hm[:],
                                        in1=bc(A1q_sb[:, :HID], [(0, 128), (1, HID)]),
                                        op=OP.mult)
                vb = nrm.tile([128, HID], f32, name="vb")
                nc.vector.tensor_tensor(out=vb[:], in0=va[:],
                                        in1=xrb[:, W:W + HID], op=OP.subtract)
                hpool = poolsb.tile([128, HID + 1], bf16, name="hpool")
                nc.vector.memset(hpool[:, HID:HID + 1], 1.0)
                nc.vector.tensor_scalar_max(hpool[:, :HID], vb[:], 0.0)
                og = poolsb.tile([128, c.G], bf16, name="og")
                nc.vector.tensor_tensor(
                    out=og[:],
                    in0=bc(batchloc_sb[:, blk:blk + 1], [(0, 128), (None, c.G)]),
                    in1=bc(ioG_sb[:], [(0, 128), (1, c.G)]),
                    op=OP.is_equal)
                nc.tensor.matmul(pooled_ps[:], og[:], hpool[:],
                                 start=(blk == 0), stop=(blk == c.NB - 1))

            edge_phase(htab1_full, h1Te, Wl1b_sb, Wr1x_sb, att1_sb, 256, 320,
                       l1_post)

            pool_sb = poolsb.tile([c.G, HID + 1], f32, name="poolsb")
            nc.vector.tensor_copy(out=pool_sb[:], in_=pooled_ps[:])
            nc.sync.dma_start(out=pool_in[:, :], in_=pool_sb[:])

        if sim_mode:
            nc.sync.dma_start(out=pool_out[:, :], in_=pool_in[:, :])
        else:
            nc.gpsimd.collective_compute(
                "AllReduce", mybir.AluOpType.add, ins=[pool_in],
                outs=[pool_out], replica_groups=RG)

        # ---------------- classifier ---------------------------------------
        with tc.tile_pool(name="cls", bufs=1) as cls, \
             tc.tile_pool(name="clsps", bufs=2, space="PSUM") as clsps:
            pall = cls.tile([c.G, HID + 1], f32)
            nc.sync.dma_start(out=pall[:], in_=pool_out[:, :])
            cnt = cls.tile([c.G, 1], f32)
            nc.vector.tensor_scalar_max(cnt[:], pall[:, HID:HID + 1], 1.0)
            rcnt = cls.tile([c.G, 1], f32)
            nc.vector.reciprocal(rcnt[:], cnt[:])
            pm = cls.tile([c.G, HID], f32)
            nc.vector.tensor_scalar_mul(pm[:], pall[:, :HID], rcnt[:])
            pmT_ps = clsps.tile([HID, c.G], f32)
            nc.tensor.transpose(pmT_ps[:], pm[:], ident32_sb[:c.G, :c.G])
            pmT = cls.tile([HID, c.G], f32)
            nc.vector.tensor_copy(out=pmT[:], in_=pmT_ps[:])
            z1_ps = clsps.tile([32, c.G], f32)
            nc.tensor.matmul(z1_ps[:], Wc1_sb[:], pmT[:], start=True, stop=True)
            z1 = cls.tile([32, c.G], f32)
            nc.scalar.activation(z1[:], z1_ps[:], AF.Relu, bias=bc1_sb[:])
            o_ps = clsps.tile([2, c.G], f32)
            nc.tensor.matmul(o_ps[:], Wc2_sb[:], z1[:], start=True, stop=True)
            o_sb = cls.tile([2, c.G], f32)
            nc.scalar.activation(o_sb[:], o_ps[:], AF.Identity, bias=bc2_sb[:])
            nc.sync.dma_start(out=out_t[:, :], in_=o_sb[:])

    nc.compile()
    return nc


# ---------------------------------------------------------------- driver ----
_BUILT = {}
LAST_RESULTS = None


def _get_program(cfg):
    key = (cfg.N, cfg.E, cfg.G, cfg.KQ)
    if key not in _BUILT:
        _BUILT[key] = build_program(cfg)
    return _BUILT[key]


def kernel(**inputs):
    from concourse import bass_utils

    x = np.asarray(inputs["x"])
    edge_index = np.asarray(inputs["edge_index"])
    N = x.shape[0]
    E = edge_index.shape[1]
    G = 64
    KQ = compute_kq(inputs)
    cfg = Cfg(N, E, G, KQ)
    per_core = host_prep(inputs, cfg)
    nc = _get_program(cfg)
    in_maps = [{k: np.ascontiguousarray(v) for k, v in m.items()} for m in per_core]
    res = bass_utils.run_bass_kernel_spmd(nc, in_maps, core_ids=list(range(NCORES)))
    global LAST_RESULTS
    LAST_RESULTS = res
    out = res.results[0]["out_t"]  # [2, G]
    return np.ascontiguousarray(out.T.astype(np.float32))

